# revision 6
# baseline (speedup 1.0000x reference)
"""Trainium2 Bass kernel for the DiagonalSSMBlock problem.

Math (per batch, sharded one batch per core over 8 cores):
    a = -exp(log_neg_real) + i*imag ; a_bar = exp(a) = r * e^{i theta}
    b_bar = ((a_bar-1)/a)[:,None] * B
    Bu_t = b_bar @ u_t                         (complex, state dim 64)
    h_t = a_bar * h_{t-1} + Bu_t               (diagonal complex scan over L)
    y_t = Re(C @ h_t) + D*u_t ; out = LN(u + y) * gamma + beta

Device decomposition (all f32):
  * u tiles [128l, 1024d] are transposed on PE (identity matmul) to feed the
    Bu matmul (contraction over d needs d on partitions).
  * Bu produced directly in scan layout [re|im states on 128 partitions, L free]
    via a packed [b_re; b_im]^T stationary operand.
  * Complex scan is rotated into a per-lane REAL damped scan:
    g_t = r*g_{t-1} + w_t with w_t = e^{-i theta t} Bu_t (elementwise rotation
    against host-precomputed cos/sin tables), h_re_t = Re(e^{i theta t} g_t).
    The real scan maps to one DVE tensor_tensor_scan per 512-wide slice.
  * Readout y = h_re^T @ C^T on PE, residual+LayerNorm fused on DVE/ACT/POOL.
"""

import numpy as np

import concourse.bass as bass
import concourse.mybir as mybir
import concourse.tile as tile
from concourse import bacc, bass_utils
from concourse.bass import MemorySpace
from concourse.masks import make_identity
from concourse.mybir import ActivationFunctionType as act
from concourse.mybir import AluOpType as alu

F32 = mybir.dt.float32
P = 128          # partitions
L = 4096         # sequence length per core
DM = 1024        # d_model
NS = 64          # d_state
LT = 512         # l-tile (scan slice, matmul moving width)
NSUB = LT // P   # 4 l-subtiles of 128 rows per l-tile
NT = L // LT     # 8 l-tiles
KC = DM // P     # 8 contraction chunks of 128
NCORES = 8
LN_EPS = 1e-5
DH = 512         # d-model half (psum bank width)


def _build_program(use_ures: bool, use_gb: bool):
    """Builds the single-core Bass/Tile program (SPMD across 8 cores).

    use_ures: residual uses a separate host-scaled input (when D != 0).
    use_gb:  apply gamma/beta via replicated tiles (when non-trivial).
    """
    nc = bacc.Bacc("TRN2", num_devices=NCORES, debug=False)

    u_d = nc.dram_tensor("u", [L, DM], F32, kind="ExternalInput").ap()
    bb_d = nc.dram_tensor("bb", [P, DM], F32, kind="ExternalInput").ap()
    ct_d = nc.dram_tensor("ct", [NS, DM], F32, kind="ExternalInput").ap()
    trig_d = nc.dram_tensor("trig", [P, L], F32, kind="ExternalInput").ap()
    rt_d = nc.dram_tensor("rt", [P, LT], F32, kind="ExternalInput").ap()
    ures_d = (
        nc.dram_tensor("ures", [L, DM], F32, kind="ExternalInput").ap()
        if use_ures
        else None
    )
    if use_gb:
        gam_d = nc.dram_tensor("gam", [P, DM], F32, kind="ExternalInput").ap()
        bet_d = nc.dram_tensor("bet", [P, DM], F32, kind="ExternalInput").ap()
    out_d = nc.dram_tensor("out", [L, DM], F32, kind="ExternalOutput").ap()

    with tile.TileContext(nc) as tc:
        with (
            tc.tile_pool(name="singles", bufs=1) as singles,
            tc.tile_pool(name="u", bufs=10) as u_pool,
            tc.tile_pool(name="ur", bufs=10) as ur_pool,
            tc.tile_pool(name="uT", bufs=3) as uT_pool,
            tc.tile_pool(name="w", bufs=2) as w_pool,
            tc.tile_pool(name="g", bufs=3) as g_pool,
            tc.tile_pool(name="h", bufs=2) as h_pool,
            tc.tile_pool(name="x", bufs=6) as x_pool,
            tc.tile_pool(name="tmp", bufs=3) as tmp_pool,
            tc.tile_pool(name="o", bufs=4) as o_pool,
            tc.tile_pool(name="st", bufs=3) as st_pool,
            tc.tile_pool(name="pt", bufs=2, space=MemorySpace.PSUM) as psum_t,
            tc.tile_pool(name="pb", bufs=2, space=MemorySpace.PSUM) as psum_b,
            tc.tile_pool(name="py", bufs=2, space=MemorySpace.PSUM) as psum_y,
        ):
            ident = singles.tile([P, P], F32)
            make_identity(nc, ident[:])
            bb_s = singles.tile([P, DM], F32)
            nc.sync.dma_start(bb_s[:], bb_d)
            ct_s = singles.tile([NS, DM], F32)
            nc.sync.dma_start(ct_s[:], ct_d)
            trig = singles.tile([P, L], F32)
            nc.sync.dma_start(trig[:], trig_d)
            rt_s = singles.tile([P, LT], F32)
            nc.sync.dma_start(rt_s[:], rt_d)
            eps_s = singles.tile([P, 1], F32)
            nc.gpsimd.memset(eps_s[:], LN_EPS)
            if use_gb:
                gam_s = singles.tile([P, DM], F32)
                nc.sync.dma_start(gam_s[:], gam_d)
                bet_s = singles.tile([P, DM], F32)
                nc.sync.dma_start(bet_s[:], bet_d)

            g_prev = None
            for it in range(NT):
                l0 = it * LT
                u_subs = []
                ur_subs = []
                for ls in range(NSUB):
                    ut = u_pool.tile([P, DM], F32, tag="u")
                    nc.sync.dma_start(ut[:], u_d[l0 + ls * P : l0 + (ls + 1) * P, :])
                    u_subs.append(ut)
                    if use_ures:
                        urt = ur_pool.tile([P, DM], F32, tag="ur")
                        nc.sync.dma_start(
                            urt[:], ures_d[l0 + ls * P : l0 + (ls + 1) * P, :]
                        )
                        ur_subs.append(urt)
                    else:
                        ur_subs.append(ut)

                # --- transpose u (PE) + Bu matmul, accumulated over d-chunks ---
                bu = psum_b.tile([P, LT], F32, tag="bu")
                for k in range(KC):
                    tp = psum_t.tile([P, LT], F32, tag="tp")
                    for ls in range(NSUB):
                        nc.tensor.matmul(
                            tp[:, ls * P : (ls + 1) * P],
                            u_subs[ls][:, k * P : (k + 1) * P],
                            ident[:],
                            start=True,
                            stop=True,
                        )
                    uT = uT_pool.tile([P, LT], F32, tag="uT")
                    nc.scalar.copy(uT[:], tp[:])
                    nc.tensor.matmul(
                        bu[:],
                        bb_s[:, k * P : (k + 1) * P],
                        uT[:],
                        start=(k == 0),
                        stop=(k == KC - 1),
                    )

                cs = trig[0:NS, l0 : l0 + LT]
                sn = trig[NS:P, l0 : l0 + LT]

                # --- pre-rotation: w = e^{-i theta t} * Bu  (DVE) ---
                w = w_pool.tile([P, LT], F32, tag="w")
                t1 = tmp_pool.tile([NS, LT], F32, tag="t1")
                t2 = tmp_pool.tile([NS, LT], F32, tag="t2")
                nc.vector.tensor_tensor(t1[:], bu[0:NS, :], cs, alu.mult)
                nc.vector.tensor_tensor(t2[:], bu[NS:P, :], sn, alu.mult)
                nc.vector.tensor_tensor(w[0:NS, :], t1[:], t2[:], alu.add)
                t3 = tmp_pool.tile([NS, LT], F32, tag="t1")
                t4 = tmp_pool.tile([NS, LT], F32, tag="t2")
                nc.vector.tensor_tensor(t3[:], bu[NS:P, :], cs, alu.mult)
                nc.vector.tensor_tensor(t4[:], bu[0:NS, :], sn, alu.mult)
                nc.vector.tensor_tensor(w[NS:P, :], t3[:], t4[:], alu.subtract)

                # --- damped real scan (DVE), chained across l-tiles ---
                g = g_pool.tile([P, LT], F32, tag="g")
                init = 0.0 if g_prev is None else g_prev[:, LT - 1 : LT]
                nc.vector.tensor_tensor_scan(
                    g[:], rt_s[:], w[:], init, alu.mult, alu.add
                )
                g_prev = g

                # --- post-rotation: h_re = cos*g_re - sin*g_im  (POOL) ---
                h = h_pool.tile([NS, LT], F32, tag="h")
                t5 = tmp_pool.tile([NS, LT], F32, tag="t1")
                t6 = tmp_pool.tile([NS, LT], F32, tag="t2")
                nc.gpsimd.tensor_tensor(t5[:], g[0:NS, :], cs, alu.mult)
                nc.gpsimd.tensor_tensor(t6[:], g[NS:P, :], sn, alu.mult)
                nc.gpsimd.tensor_tensor(h[:], t5[:], t6[:], alu.subtract)

                # --- readout + residual + LN stats per l-subtile ---
                sx = st_pool.tile([P, 2 * NSUB], F32, tag="sx")
                sq = st_pool.tile([P, NSUB], F32, tag="sq")
                x_list = []
                for ls in range(NSUB):
                    y = psum_y.tile([P, DM], F32, tag="y")
                    for dh in range(2):
                        nc.tensor.matmul(
                            y[:, dh * DH : (dh + 1) * DH],
                            h[:, ls * P : (ls + 1) * P],
                            ct_s[:, dh * DH : (dh + 1) * DH],
                            start=True,
                            stop=True,
                        )
                    x = x_pool.tile([P, DM], F32, tag="x")
                    for dh in range(2):
                        nc.vector.scalar_tensor_tensor(
                            x[:, dh * DH : (dh + 1) * DH],
                            y[:, dh * DH : (dh + 1) * DH],
                            1.0,
                            ur_subs[ls][:, dh * DH : (dh + 1) * DH],
                            alu.mult,
                            alu.add,
                            accum_out=sx[:, 2 * ls + dh : 2 * ls + dh + 1],
                        )
                    sqs = tmp_pool.tile([P, DM], F32, tag="sqs")
                    nc.scalar.activation(
                        sqs[:], x[:], act.Square, accum_out=sq[:, ls : ls + 1]
                    )
                    x_list.append(x)

                # --- LN stats for the 4 l-subtiles ---
                mu = st_pool.tile([P, NSUB], F32, tag="mu")
                nc.vector.tensor_tensor(
                    mu[:], sx[:, 0 : 2 * NSUB : 2], sx[:, 1 : 2 * NSUB : 2], alu.add
                )
                nc.scalar.mul(mu[:], mu[:], 1.0 / DM)
                ex2 = st_pool.tile([P, NSUB], F32, tag="ex2")
                nc.scalar.mul(ex2[:], sq[:], 1.0 / DM)
                var = st_pool.tile([P, NSUB], F32, tag="var")
                nc.vector.tensor_tensor(var[:], mu[:], mu[:], alu.mult)
                nc.vector.tensor_tensor(var[:], ex2[:], var[:], alu.subtract)
                sd = st_pool.tile([P, NSUB], F32, tag="sd")
                nc.scalar.activation(sd[:], var[:], act.Sqrt, bias=eps_s[:, 0:1])
                rstd = st_pool.tile([P, NSUB], F32, tag="rstd")
                nc.vector.reciprocal(rstd[:], sd[:])

                # --- normalize (POOL) + store ---
                for ls in range(NSUB):
                    o = o_pool.tile([P, DM], F32, tag="o")
                    nc.gpsimd.tensor_scalar(
                        o[:],
                        x_list[ls][:],
                        mu[:, ls : ls + 1],
                        rstd[:, ls : ls + 1],
                        alu.subtract,
                        alu.mult,
                    )
                    if use_gb:
                        nc.vector.tensor_tensor(o[:], o[:], gam_s[:], alu.mult)
                        nc.vector.tensor_tensor(o[:], o[:], bet_s[:], alu.add)
                    nc.sync.dma_start(
                        out_d[l0 + ls * P : l0 + (ls + 1) * P, :], o[:]
                    )
    nc.compile()
    return nc


def _host_params(log_neg_real, imag, B_mat, C_mat):
    lnr = np.asarray(log_neg_real, np.float64)
    im = np.asarray(imag, np.float64)
    a = -np.exp(lnr) + 1j * im
    a_bar = np.exp(a)
    r = np.abs(a_bar)
    b_bar = ((a_bar - 1.0) / a)[:, None] * np.asarray(B_mat, np.float64)
    b_re = np.real(b_bar).astype(np.float32)
    b_im = np.imag(b_bar).astype(np.float32)
    # packed stationary operand for the Bu matmul: [K=d, M=128(re|im)] laid out
    # in SBUF as [128 partitions, KC*128] with chunk k at columns k*128:(k+1)*128
    bbT = np.concatenate([b_re, b_im], axis=0).T  # (DM, 128)
    bb = np.ascontiguousarray(
        bbT.reshape(KC, P, P).transpose(1, 0, 2).reshape(P, DM)
    )
    ct = np.ascontiguousarray(np.asarray(C_mat, np.float32).T)  # (NS, DM)
    t = np.arange(L, dtype=np.float64)
    ang = (im[:, None] * t[None, :]) % (2 * np.pi)
    cosT = np.cos(ang).astype(np.float32)
    sinT = np.sin(ang).astype(np.float32)
    trig = np.ascontiguousarray(np.concatenate([cosT, sinT], axis=0))  # (128, L)
    rfull = np.concatenate([r, r]).astype(np.float32)
    rt = np.ascontiguousarray(np.broadcast_to(rfull[:, None], (P, LT)))
    return bb, ct, trig, rt


def kernel(u, log_neg_real, imag, B_mat, C_mat, D, gamma, beta, _cache={}):
    u = np.ascontiguousarray(np.asarray(u, np.float32))
    Dv = np.asarray(D, np.float32)
    gam = np.asarray(gamma, np.float32)
    bet = np.asarray(beta, np.float32)
    use_ures = bool(np.any(Dv != 0.0))
    use_gb = bool(np.any(gam != 1.0) or np.any(bet != 0.0))

    bb, ct, trig, rt = _host_params(log_neg_real, imag, B_mat, C_mat)

    key = (use_ures, use_gb)
    if key not in _cache:
        _cache[key] = _build_program(use_ures, use_gb)
    nc = _cache[key]

    shared = {"bb": bb, "ct": ct, "trig": trig, "rt": rt}
    if use_gb:
        shared["gam"] = np.ascontiguousarray(
            np.broadcast_to(gam[None, :], (P, DM)).astype(np.float32)
        )
        shared["bet"] = np.ascontiguousarray(
            np.broadcast_to(bet[None, :], (P, DM)).astype(np.float32)
        )
    in_maps = []
    for b in range(NCORES):
        m = dict(shared)
        m["u"] = np.ascontiguousarray(u[b])
        if use_ures:
            m["ures"] = np.ascontiguousarray(u[b] * (1.0 + Dv)[None, :])
        in_maps.append(m)

    res = bass_utils.run_bass_kernel_spmd(nc, in_maps, core_ids=list(range(NCORES)))
    return np.stack([r["out"] for r in res.results], axis=0)


# revision 14
# speedup vs baseline: 2.2593x; 2.2593x over previous
"""Trainium2 Bass kernel for the DiagonalSSMBlock problem.

Math (per batch, sharded one batch per core over 8 cores):
    a = -exp(log_neg_real) + i*imag ; a_bar = exp(a) = r * e^{i theta}
    b_bar = ((a_bar-1)/a)[:,None] * B
    Bu_t = b_bar @ u_t                         (complex, state dim 64)
    h_t = a_bar * h_{t-1} + Bu_t               (diagonal complex scan over L)
    y_t = Re(C @ h_t) + D*u_t ; out = LN(u + y) * gamma + beta

Device decomposition (all f32):
  * u tiles [128l, 1024d] are transposed on PE (identity matmul) to feed the
    Bu matmul (contraction over d needs d on partitions).
  * Bu produced directly in scan layout [re|im states on 128 partitions, L free]
    via a packed [b_re; b_im]^T stationary operand.
  * Complex scan is rotated into a per-lane REAL damped scan:
    g_t = r*g_{t-1} + w_t with w_t = e^{-i theta t} Bu_t (elementwise rotation
    against host-precomputed cos/sin tables), h_re_t = Re(e^{i theta t} g_t).
    The real scan maps to one DVE tensor_tensor_scan per 512-wide slice.
  * Readout y = h_re^T @ C^T on PE, residual+LayerNorm fused on DVE/ACT/POOL.
"""

import numpy as np

import concourse.bass as bass
import concourse.mybir as mybir
import concourse.tile as tile
from concourse import bacc, bass_utils
from concourse.bass import MemorySpace
from concourse.masks import make_identity
from concourse.mybir import ActivationFunctionType as act
from concourse.mybir import AluOpType as alu

F32 = mybir.dt.float32
P = 128          # partitions
L = 4096         # sequence length per core
DM = 1024        # d_model
NS = 64          # d_state
LT = 512         # l-tile (scan slice, matmul moving width)
NSUB = LT // P   # 4 l-subtiles of 128 rows per l-tile
NT = L // LT     # 8 l-tiles
KC = DM // P     # 8 contraction chunks of 128
NCORES = 8
LN_EPS = 1e-5
DH = 512         # d-model half (psum bank width)


def _build_program(use_ures: bool, use_gb: bool):
    """Builds the single-core Bass/Tile program (SPMD across 8 cores).

    use_ures: residual uses a separate host-scaled input (when D != 0).
    use_gb:  apply gamma/beta via replicated tiles (when non-trivial).
    """
    nc = bacc.Bacc("TRN2", num_devices=NCORES, debug=False)

    u_d = nc.dram_tensor("u", [L, DM], F32, kind="ExternalInput").ap()
    bb_d = nc.dram_tensor("bb", [P, DM], F32, kind="ExternalInput").ap()
    ct_d = nc.dram_tensor("ct", [NS, DM], F32, kind="ExternalInput").ap()
    trig_d = nc.dram_tensor("trig", [P, L], F32, kind="ExternalInput").ap()
    trigb_d = nc.dram_tensor("trigb", [P, L], F32, kind="ExternalInput").ap()
    rt_d = nc.dram_tensor("rt", [P, LT], F32, kind="ExternalInput").ap()
    ures_d = (
        nc.dram_tensor("ures", [L, DM], F32, kind="ExternalInput").ap()
        if use_ures
        else None
    )
    if use_gb:
        gam_d = nc.dram_tensor("gam", [P, DM], F32, kind="ExternalInput").ap()
        bet_d = nc.dram_tensor("bet", [P, DM], F32, kind="ExternalInput").ap()
    out_d = nc.dram_tensor("out", [L, DM], F32, kind="ExternalOutput").ap()

    with tile.TileContext(nc) as tc:
        with (
            tc.tile_pool(name="singles", bufs=1) as singles,
            tc.tile_pool(name="u", bufs=10) as u_pool,
            tc.tile_pool(name="ur", bufs=10) as ur_pool,
            tc.tile_pool(name="uT", bufs=3) as uT_pool,
            tc.tile_pool(name="w", bufs=2) as w_pool,
            tc.tile_pool(name="g", bufs=3) as g_pool,
            tc.tile_pool(name="h", bufs=2) as h_pool,
            tc.tile_pool(name="x", bufs=6) as x_pool,
            tc.tile_pool(name="tmp", bufs=3) as tmp_pool,
            tc.tile_pool(name="o", bufs=4) as o_pool,
            tc.tile_pool(name="st", bufs=3) as st_pool,
            tc.tile_pool(name="pt", bufs=2, space=MemorySpace.PSUM) as psum_t,
            tc.tile_pool(name="pb", bufs=2, space=MemorySpace.PSUM) as psum_b,
            tc.tile_pool(name="py", bufs=2, space=MemorySpace.PSUM) as psum_y,
        ):
            ident = singles.tile([P, P], F32)
            make_identity(nc, ident[:])
            bb_s = singles.tile([P, DM], F32)
            nc.sync.dma_start(bb_s[:], bb_d)
            ct_s = singles.tile([NS, DM], F32)
            nc.sync.dma_start(ct_s[:], ct_d)
            trig = singles.tile([P, L], F32)
            nc.sync.dma_start(trig[:], trig_d)
            trigb = singles.tile([P, L], F32)
            nc.sync.dma_start(trigb[:], trigb_d)
            rt_s = singles.tile([P, LT], F32)
            nc.sync.dma_start(rt_s[:], rt_d)
            eps_s = singles.tile([P, 1], F32)
            nc.gpsimd.memset(eps_s[:], LN_EPS)
            if use_gb:
                gam_s = singles.tile([P, DM], F32)
                nc.sync.dma_start(gam_s[:], gam_d)
                bet_s = singles.tile([P, DM], F32)
                nc.sync.dma_start(bet_s[:], bet_d)

            g_prev = None
            for it in range(NT):
                l0 = it * LT
                u_subs = []
                ur_subs = []
                for ls in range(NSUB):
                    ut = u_pool.tile([P, DM], F32, tag="u")
                    nc.sync.dma_start(ut[:], u_d[l0 + ls * P : l0 + (ls + 1) * P, :])
                    u_subs.append(ut)
                    if use_ures:
                        urt = ur_pool.tile([P, DM], F32, tag="ur")
                        nc.sync.dma_start(
                            urt[:], ures_d[l0 + ls * P : l0 + (ls + 1) * P, :]
                        )
                        ur_subs.append(urt)
                    else:
                        ur_subs.append(ut)

                # --- transpose u (PE) + Bu matmul, accumulated over d-chunks ---
                bu = psum_b.tile([P, LT], F32, tag="bu")
                for k in range(KC):
                    tp = psum_t.tile([P, LT], F32, tag="tp")
                    for ls in range(NSUB):
                        nc.tensor.matmul(
                            tp[:, ls * P : (ls + 1) * P],
                            u_subs[ls][:, k * P : (k + 1) * P],
                            ident[:],
                            start=True,
                            stop=True,
                        )
                    uT = uT_pool.tile([P, LT], F32, tag="uT")
                    nc.scalar.copy(uT[:], tp[:])
                    nc.tensor.matmul(
                        bu[:],
                        bb_s[:, k * P : (k + 1) * P],
                        uT[:],
                        start=(k == 0),
                        stop=(k == KC - 1),
                    )

                # trig: cos on parts 0-63, sin on 64-127; trigb: swapped halves.
                # SBUF+SBUF operands must share base partition (birverifier);
                # out may target either half.
                cs_lo = trig[0:NS, l0 : l0 + LT]
                sn_hi = trig[NS:P, l0 : l0 + LT]
                sn_lo = trigb[0:NS, l0 : l0 + LT]
                cs_hi = trigb[NS:P, l0 : l0 + LT]

                # --- pre-rotation: w = e^{-i theta t} * Bu  (DVE, SBUF-staged) ---
                bs = w_pool.tile([P, LT], F32, tag="bs")
                nc.scalar.copy(bs[:], bu[:])
                w = w_pool.tile([P, LT], F32, tag="w")
                t1 = tmp_pool.tile([NS, LT], F32, tag="t1")
                t2 = tmp_pool.tile([NS, LT], F32, tag="t2")
                nc.vector.tensor_tensor(t1[:], bs[0:NS, :], cs_lo, alu.mult)
                nc.vector.tensor_tensor(t2[:], bs[NS:P, :], sn_hi, alu.mult)
                nc.vector.tensor_tensor(w[0:NS, :], t1[:], t2[:], alu.add)
                t3 = tmp_pool.tile([NS, LT], F32, tag="t1")
                t4 = tmp_pool.tile([NS, LT], F32, tag="t2")
                nc.vector.tensor_tensor(t3[:], bs[NS:P, :], cs_hi, alu.mult)
                nc.vector.tensor_tensor(t4[:], bs[0:NS, :], sn_lo, alu.mult)
                nc.vector.tensor_tensor(w[NS:P, :], t3[:], t4[:], alu.subtract)

                # --- damped real scan (DVE), chained across l-tiles ---
                g = g_pool.tile([P, LT], F32, tag="g")
                init = 0.0 if g_prev is None else g_prev[:, LT - 1 : LT]
                nc.vector.tensor_tensor_scan(
                    g[:], rt_s[:], w[:], init, alu.mult, alu.add
                )
                g_prev = g

                # --- post-rotation: h_re = cos*g_re - sin*g_im  (POOL) ---
                h = h_pool.tile([NS, LT], F32, tag="h")
                t5 = tmp_pool.tile([NS, LT], F32, tag="t1")
                t6 = tmp_pool.tile([NS, LT], F32, tag="t2")
                nc.gpsimd.tensor_tensor(t5[:], g[0:NS, :], cs_lo, alu.mult)
                nc.gpsimd.tensor_tensor(t6[:], g[NS:P, :], sn_hi, alu.mult)
                nc.gpsimd.tensor_tensor(h[:], t5[:], t6[:], alu.subtract)

                # --- readout + residual + LN stats per l-subtile ---
                sx = st_pool.tile([P, 2 * NSUB], F32, tag="sx")
                sq = st_pool.tile([P, NSUB], F32, tag="sq")
                x_list = []
                for ls in range(NSUB):
                    y = psum_y.tile([P, DM], F32, tag="y")
                    for dh in range(2):
                        nc.tensor.matmul(
                            y[:, dh * DH : (dh + 1) * DH],
                            h[:, ls * P : (ls + 1) * P],
                            ct_s[:, dh * DH : (dh + 1) * DH],
                            start=True,
                            stop=True,
                        )
                    x = x_pool.tile([P, DM], F32, tag="x")
                    for dh in range(2):
                        nc.vector.scalar_tensor_tensor(
                            x[:, dh * DH : (dh + 1) * DH],
                            y[:, dh * DH : (dh + 1) * DH],
                            1.0,
                            ur_subs[ls][:, dh * DH : (dh + 1) * DH],
                            alu.mult,
                            alu.add,
                            accum_out=sx[:, 2 * ls + dh : 2 * ls + dh + 1],
                        )
                    sqs = tmp_pool.tile([P, DM], F32, tag="sqs")
                    nc.scalar.activation(
                        sqs[:], x[:], act.Square, accum_out=sq[:, ls : ls + 1]
                    )
                    x_list.append(x)

                # --- LN stats for the 4 l-subtiles ---
                mu = st_pool.tile([P, NSUB], F32, tag="mu")
                nc.vector.tensor_tensor(
                    mu[:], sx[:, 0 : 2 * NSUB : 2], sx[:, 1 : 2 * NSUB : 2], alu.add
                )
                nc.scalar.mul(mu[:], mu[:], 1.0 / DM)
                ex2 = st_pool.tile([P, NSUB], F32, tag="ex2")
                nc.scalar.mul(ex2[:], sq[:], 1.0 / DM)
                var = st_pool.tile([P, NSUB], F32, tag="var")
                nc.vector.tensor_tensor(var[:], mu[:], mu[:], alu.mult)
                nc.vector.tensor_tensor(var[:], ex2[:], var[:], alu.subtract)
                sd = st_pool.tile([P, NSUB], F32, tag="sd")
                nc.scalar.activation(sd[:], var[:], act.Sqrt, bias=eps_s[:, 0:1])
                rstd = st_pool.tile([P, NSUB], F32, tag="rstd")
                nc.vector.reciprocal(rstd[:], sd[:])

                # --- normalize (POOL) + store ---
                for ls in range(NSUB):
                    o = o_pool.tile([P, DM], F32, tag="o")
                    nc.vector.tensor_scalar(
                        o[:],
                        x_list[ls][:],
                        mu[:, ls : ls + 1],
                        rstd[:, ls : ls + 1],
                        alu.subtract,
                        alu.mult,
                    )
                    if use_gb:
                        nc.vector.tensor_tensor(o[:], o[:], gam_s[:], alu.mult)
                        nc.vector.tensor_tensor(o[:], o[:], bet_s[:], alu.add)
                    nc.sync.dma_start(
                        out_d[l0 + ls * P : l0 + (ls + 1) * P, :], o[:]
                    )
    nc.compile()
    return nc


def _host_params(log_neg_real, imag, B_mat, C_mat):
    lnr = np.asarray(log_neg_real, np.float64)
    im = np.asarray(imag, np.float64)
    a = -np.exp(lnr) + 1j * im
    a_bar = np.exp(a)
    r = np.abs(a_bar)
    b_bar = ((a_bar - 1.0) / a)[:, None] * np.asarray(B_mat, np.float64)
    b_re = np.real(b_bar).astype(np.float32)
    b_im = np.imag(b_bar).astype(np.float32)
    # packed stationary operand for the Bu matmul: [K=d, M=128(re|im)] laid out
    # in SBUF as [128 partitions, KC*128] with chunk k at columns k*128:(k+1)*128
    bbT = np.concatenate([b_re, b_im], axis=0).T  # (DM, 128)
    bb = np.ascontiguousarray(
        bbT.reshape(KC, P, P).transpose(1, 0, 2).reshape(P, DM)
    )
    ct = np.ascontiguousarray(np.asarray(C_mat, np.float32).T)  # (NS, DM)
    t = np.arange(L, dtype=np.float64)
    ang = (im[:, None] * t[None, :]) % (2 * np.pi)
    cosT = np.cos(ang).astype(np.float32)
    sinT = np.sin(ang).astype(np.float32)
    trig = np.ascontiguousarray(np.concatenate([cosT, sinT], axis=0))  # (128, L)
    trigb = np.ascontiguousarray(np.concatenate([sinT, cosT], axis=0))
    rfull = np.concatenate([r, r]).astype(np.float32)
    rt = np.ascontiguousarray(np.broadcast_to(rfull[:, None], (P, LT)))
    return bb, ct, trig, trigb, rt


def kernel(u, log_neg_real, imag, B_mat, C_mat, D, gamma, beta, _cache={}):
    u = np.ascontiguousarray(np.asarray(u, np.float32))
    Dv = np.asarray(D, np.float32)
    gam = np.asarray(gamma, np.float32)
    bet = np.asarray(beta, np.float32)
    use_ures = bool(np.any(Dv != 0.0))
    use_gb = bool(np.any(gam != 1.0) or np.any(bet != 0.0))

    bb, ct, trig, trigb, rt = _host_params(log_neg_real, imag, B_mat, C_mat)

    key = (use_ures, use_gb)
    if key not in _cache:
        _cache[key] = _build_program(use_ures, use_gb)
    nc = _cache[key]

    shared = {"bb": bb, "ct": ct, "trig": trig, "trigb": trigb, "rt": rt}
    if use_gb:
        shared["gam"] = np.ascontiguousarray(
            np.broadcast_to(gam[None, :], (P, DM)).astype(np.float32)
        )
        shared["bet"] = np.ascontiguousarray(
            np.broadcast_to(bet[None, :], (P, DM)).astype(np.float32)
        )
    in_maps = []
    for b in range(NCORES):
        m = dict(shared)
        m["u"] = np.ascontiguousarray(u[b])
        if use_ures:
            m["ures"] = np.ascontiguousarray(u[b] * (1.0 + Dv)[None, :])
        in_maps.append(m)

    res = bass_utils.run_bass_kernel_spmd(nc, in_maps, core_ids=list(range(NCORES)))
    return np.stack([r["out"] for r in res.results], axis=0)


# revision 16
# speedup vs baseline: 2.5018x; 1.1073x over previous
"""Trainium2 Bass kernel for the DiagonalSSMBlock problem.

Math (per batch, sharded one batch per core over 8 cores):
    a = -exp(log_neg_real) + i*imag ; a_bar = exp(a) = r * e^{i theta}
    b_bar = ((a_bar-1)/a)[:,None] * B
    Bu_t = b_bar @ u_t                         (complex, state dim 64)
    h_t = a_bar * h_{t-1} + Bu_t               (diagonal complex scan over L)
    y_t = Re(C @ h_t) + D*u_t ; out = LN(u + y) * gamma + beta

Device decomposition:
  * The Bu matmul contracts over d_model, so it consumes u in transposed
    layout. fp32 matmuls on the PE run in LOW_HIGH double-pass mode (~4.7x
    bf16 cost), so u is shipped as a host-precomputed transposed bf16 hi/lo
    pair and each matmul runs as 3 accumulating bf16 matmuls
    (hi*hi + lo*hi + hi*lo), recovering ~fp32 accuracy at bf16 speed.
  * Bu lands directly in scan layout [re|im states on 128 partitions, L free]
    via a packed [b_re; b_im]^T stationary operand.
  * The complex scan is rotated into a per-lane REAL damped scan:
    g_t = r*g_{t-1} + w_t with w_t = e^{-i theta t} Bu_t (elementwise
    rotation against host cos/sin tables), h_re_t = Re(e^{i theta t} g_t).
    The real scan maps to one DVE tensor_tensor_scan per 512-wide slice,
    chained via its initial value.
  * Readout y = h_re^T @ C^T (compensated bf16), residual + LayerNorm fused
    on DVE/ACT: scalar_tensor_tensor computes x=y+u and accumulates sum(x),
    ACT Square accumulates sum(x^2), DVE tensor_scalar applies (x-mu)*rstd.
"""

import numpy as np

import concourse.mybir as mybir
import concourse.tile as tile
from concourse import bacc, bass_utils
from concourse.bass import MemorySpace
from concourse.mybir import ActivationFunctionType as act
from concourse.mybir import AluOpType as alu

F32 = mybir.dt.float32
BF16 = mybir.dt.bfloat16
P = 128          # partitions
L = 4096         # sequence length per core
DM = 1024        # d_model
NS = 64          # d_state
LT = 512         # l-tile (scan slice, matmul moving width)
NSUB = LT // P   # 4 l-subtiles of 128 rows per l-tile
NT = L // LT     # 8 l-tiles
KC = DM // P     # 8 contraction chunks of 128
NCORES = 8
LN_EPS = 1e-5
DH = 512         # d-model half (psum bank width)


def _build_program(use_ures: bool, use_gb: bool):
    """Builds the single-core Bass/Tile program (SPMD across 8 cores).

    use_ures: residual uses a separate host-scaled input (when D != 0).
    use_gb:  apply gamma/beta via replicated tiles (when non-trivial).
    """
    nc = bacc.Bacc("TRN2", num_devices=NCORES, debug=False)

    u_d = nc.dram_tensor("u", [L, DM], F32, kind="ExternalInput").ap()
    uth_d = nc.dram_tensor("uth", [DM, L], BF16, kind="ExternalInput").ap()
    utl_d = nc.dram_tensor("utl", [DM, L], BF16, kind="ExternalInput").ap()
    bbh_d = nc.dram_tensor("bbh", [P, DM], BF16, kind="ExternalInput").ap()
    bbl_d = nc.dram_tensor("bbl", [P, DM], BF16, kind="ExternalInput").ap()
    cth_d = nc.dram_tensor("cth", [NS, DM], BF16, kind="ExternalInput").ap()
    ctl_d = nc.dram_tensor("ctl", [NS, DM], BF16, kind="ExternalInput").ap()
    trig_d = nc.dram_tensor("trig", [P, L], F32, kind="ExternalInput").ap()
    trigb_d = nc.dram_tensor("trigb", [P, L], F32, kind="ExternalInput").ap()
    rt_d = nc.dram_tensor("rt", [P, LT], F32, kind="ExternalInput").ap()
    ures_d = (
        nc.dram_tensor("ures", [L, DM], F32, kind="ExternalInput").ap()
        if use_ures
        else None
    )
    if use_gb:
        gam_d = nc.dram_tensor("gam", [P, DM], F32, kind="ExternalInput").ap()
        bet_d = nc.dram_tensor("bet", [P, DM], F32, kind="ExternalInput").ap()
    out_d = nc.dram_tensor("out", [L, DM], F32, kind="ExternalOutput").ap()

    with tile.TileContext(nc) as tc:
        with (
            tc.tile_pool(name="singles", bufs=1) as singles,
            tc.tile_pool(name="u", bufs=10) as u_pool,
            tc.tile_pool(name="ur", bufs=10) as ur_pool,
            tc.tile_pool(name="ut", bufs=4) as ut_pool,
            tc.tile_pool(name="w", bufs=2) as w_pool,
            tc.tile_pool(name="g", bufs=3) as g_pool,
            tc.tile_pool(name="h", bufs=2) as h_pool,
            tc.tile_pool(name="x", bufs=6) as x_pool,
            tc.tile_pool(name="tmp", bufs=3) as tmp_pool,
            tc.tile_pool(name="o", bufs=4) as o_pool,
            tc.tile_pool(name="st", bufs=3) as st_pool,
            tc.tile_pool(name="pb", bufs=2, space=MemorySpace.PSUM) as psum_b,
            tc.tile_pool(name="py", bufs=3, space=MemorySpace.PSUM) as psum_y,
        ):
            bbh_s = singles.tile([P, DM], BF16)
            nc.sync.dma_start(bbh_s[:], bbh_d)
            bbl_s = singles.tile([P, DM], BF16)
            nc.sync.dma_start(bbl_s[:], bbl_d)
            cth_s = singles.tile([NS, DM], BF16)
            nc.sync.dma_start(cth_s[:], cth_d)
            ctl_s = singles.tile([NS, DM], BF16)
            nc.sync.dma_start(ctl_s[:], ctl_d)
            trig = singles.tile([P, L], F32)
            nc.sync.dma_start(trig[:], trig_d)
            trigb = singles.tile([P, L], F32)
            nc.sync.dma_start(trigb[:], trigb_d)
            rt_s = singles.tile([P, LT], F32)
            nc.sync.dma_start(rt_s[:], rt_d)
            eps_s = singles.tile([P, 1], F32)
            nc.gpsimd.memset(eps_s[:], LN_EPS)
            if use_gb:
                gam_s = singles.tile([P, DM], F32)
                nc.sync.dma_start(gam_s[:], gam_d)
                bet_s = singles.tile([P, DM], F32)
                nc.sync.dma_start(bet_s[:], bet_d)

            g_prev = None
            for it in range(NT):
                l0 = it * LT
                u_subs = []
                ur_subs = []
                for ls in range(NSUB):
                    ut = u_pool.tile([P, DM], F32, tag="u")
                    nc.sync.dma_start(ut[:], u_d[l0 + ls * P : l0 + (ls + 1) * P, :])
                    u_subs.append(ut)
                    if use_ures:
                        urt = ur_pool.tile([P, DM], F32, tag="ur")
                        nc.sync.dma_start(
                            urt[:], ures_d[l0 + ls * P : l0 + (ls + 1) * P, :]
                        )
                        ur_subs.append(urt)
                    else:
                        ur_subs.append(ut)

                # --- Bu matmul over d-chunks, compensated bf16 ---
                bu = psum_b.tile([P, LT], F32, tag="bu")
                n_mm = 3 * KC
                mi = 0
                for k in range(KC):
                    th = ut_pool.tile([P, LT], BF16, tag="uth")
                    nc.sync.dma_start(
                        th[:], uth_d[k * P : (k + 1) * P, l0 : l0 + LT]
                    )
                    tl = ut_pool.tile([P, LT], BF16, tag="utl")
                    nc.sync.dma_start(
                        tl[:], utl_d[k * P : (k + 1) * P, l0 : l0 + LT]
                    )
                    for lhsT, rhs in (
                        (bbh_s, th),
                        (bbl_s, th),
                        (bbh_s, tl),
                    ):
                        nc.tensor.matmul(
                            bu[:],
                            lhsT[:, k * P : (k + 1) * P],
                            rhs[:],
                            start=(mi == 0),
                            stop=(mi == n_mm - 1),
                        )
                        mi += 1

                # trig: cos on parts 0-63, sin on 64-127; trigb: swapped halves.
                # SBUF+SBUF operands must share base partition (birverifier);
                # out may target either half.
                cs_lo = trig[0:NS, l0 : l0 + LT]
                sn_hi = trig[NS:P, l0 : l0 + LT]
                sn_lo = trigb[0:NS, l0 : l0 + LT]
                cs_hi = trigb[NS:P, l0 : l0 + LT]

                # --- pre-rotation: w = e^{-i theta t} * Bu  (DVE, SBUF-staged) ---
                bs = w_pool.tile([P, LT], F32, tag="bs")
                nc.scalar.copy(bs[:], bu[:])
                w = w_pool.tile([P, LT], F32, tag="w")
                t1 = tmp_pool.tile([NS, LT], F32, tag="t1")
                t2 = tmp_pool.tile([NS, LT], F32, tag="t2")
                nc.vector.tensor_tensor(t1[:], bs[0:NS, :], cs_lo, alu.mult)
                nc.vector.tensor_tensor(t2[:], bs[NS:P, :], sn_hi, alu.mult)
                nc.vector.tensor_tensor(w[0:NS, :], t1[:], t2[:], alu.add)
                t3 = tmp_pool.tile([NS, LT], F32, tag="t1")
                t4 = tmp_pool.tile([NS, LT], F32, tag="t2")
                nc.vector.tensor_tensor(t3[:], bs[NS:P, :], cs_hi, alu.mult)
                nc.vector.tensor_tensor(t4[:], bs[0:NS, :], sn_lo, alu.mult)
                nc.vector.tensor_tensor(w[NS:P, :], t3[:], t4[:], alu.subtract)

                # --- damped real scan (DVE), chained across l-tiles ---
                g = g_pool.tile([P, LT], F32, tag="g")
                init = 0.0 if g_prev is None else g_prev[:, LT - 1 : LT]
                nc.vector.tensor_tensor_scan(
                    g[:], rt_s[:], w[:], init, alu.mult, alu.add
                )
                g_prev = g

                # --- post-rotation: h_re = cos*g_re - sin*g_im  (POOL) ---
                h = h_pool.tile([NS, LT], F32, tag="h")
                t5 = tmp_pool.tile([NS, LT], F32, tag="t1")
                t6 = tmp_pool.tile([NS, LT], F32, tag="t2")
                nc.gpsimd.tensor_tensor(t5[:], g[0:NS, :], cs_lo, alu.mult)
                nc.gpsimd.tensor_tensor(t6[:], g[NS:P, :], sn_hi, alu.mult)
                nc.gpsimd.tensor_tensor(h[:], t5[:], t6[:], alu.subtract)
                # bf16 hi/lo split of h for the compensated readout matmul
                hh = h_pool.tile([NS, LT], BF16, tag="hh")
                nc.vector.tensor_copy(hh[:], h[:])
                hl = h_pool.tile([NS, LT], BF16, tag="hl")
                nc.vector.scalar_tensor_tensor(
                    hl[:], h[:], 1.0, hh[:], alu.mult, alu.subtract
                )

                # --- readout + residual + LN stats per l-subtile ---
                sx = st_pool.tile([P, 2 * NSUB], F32, tag="sx")
                sq = st_pool.tile([P, NSUB], F32, tag="sq")
                x_list = []
                for ls in range(NSUB):
                    y = psum_y.tile([P, DM], F32, tag="y")
                    for dh in range(2):
                        for si, (lhsT, rhs) in enumerate(
                            ((hh, cth_s), (hl, cth_s), (hh, ctl_s))
                        ):
                            nc.tensor.matmul(
                                y[:, dh * DH : (dh + 1) * DH],
                                lhsT[:, ls * P : (ls + 1) * P],
                                rhs[:, dh * DH : (dh + 1) * DH],
                                start=(si == 0),
                                stop=(si == 2),
                            )
                    x = x_pool.tile([P, DM], F32, tag="x")
                    for dh in range(2):
                        nc.vector.scalar_tensor_tensor(
                            x[:, dh * DH : (dh + 1) * DH],
                            y[:, dh * DH : (dh + 1) * DH],
                            1.0,
                            ur_subs[ls][:, dh * DH : (dh + 1) * DH],
                            alu.mult,
                            alu.add,
                            accum_out=sx[:, 2 * ls + dh : 2 * ls + dh + 1],
                        )
                    sqs = tmp_pool.tile([P, DM], F32, tag="sqs")
                    nc.scalar.activation(
                        sqs[:], x[:], act.Square, accum_out=sq[:, ls : ls + 1]
                    )
                    x_list.append(x)

                # --- LN stats for the 4 l-subtiles ---
                mu = st_pool.tile([P, NSUB], F32, tag="mu")
                nc.vector.tensor_tensor(
                    mu[:], sx[:, 0 : 2 * NSUB : 2], sx[:, 1 : 2 * NSUB : 2], alu.add
                )
                nc.scalar.mul(mu[:], mu[:], 1.0 / DM)
                ex2 = st_pool.tile([P, NSUB], F32, tag="ex2")
                nc.scalar.mul(ex2[:], sq[:], 1.0 / DM)
                var = st_pool.tile([P, NSUB], F32, tag="var")
                nc.vector.tensor_tensor(var[:], mu[:], mu[:], alu.mult)
                nc.vector.tensor_tensor(var[:], ex2[:], var[:], alu.subtract)
                sd = st_pool.tile([P, NSUB], F32, tag="sd")
                nc.scalar.activation(sd[:], var[:], act.Sqrt, bias=eps_s[:, 0:1])
                rstd = st_pool.tile([P, NSUB], F32, tag="rstd")
                nc.vector.reciprocal(rstd[:], sd[:])

                # --- normalize (DVE tensor_scalar) + store ---
                for ls in range(NSUB):
                    o = o_pool.tile([P, DM], F32, tag="o")
                    nc.vector.tensor_scalar(
                        o[:],
                        x_list[ls][:],
                        mu[:, ls : ls + 1],
                        rstd[:, ls : ls + 1],
                        alu.subtract,
                        alu.mult,
                    )
                    if use_gb:
                        nc.vector.tensor_tensor(o[:], o[:], gam_s[:], alu.mult)
                        nc.vector.tensor_tensor(o[:], o[:], bet_s[:], alu.add)
                    nc.sync.dma_start(
                        out_d[l0 + ls * P : l0 + (ls + 1) * P, :], o[:]
                    )
    nc.compile()
    return nc


def _split_bf16(x):
    hi = x.astype(np.float32).astype(ml_bf16)
    lo = (x.astype(np.float32) - hi.astype(np.float32)).astype(ml_bf16)
    return hi, lo


try:
    import ml_dtypes

    ml_bf16 = ml_dtypes.bfloat16
except ImportError:  # pragma: no cover
    ml_bf16 = None


def _host_params(log_neg_real, imag, B_mat, C_mat):
    lnr = np.asarray(log_neg_real, np.float64)
    im = np.asarray(imag, np.float64)
    a = -np.exp(lnr) + 1j * im
    a_bar = np.exp(a)
    r = np.abs(a_bar)
    b_bar = ((a_bar - 1.0) / a)[:, None] * np.asarray(B_mat, np.float64)
    b_re = np.real(b_bar).astype(np.float32)
    b_im = np.imag(b_bar).astype(np.float32)
    # packed stationary operand for the Bu matmul: [K=d, M=128(re|im)] laid out
    # in SBUF as [128 partitions, KC*128] with chunk k at columns k*128:(k+1)*128
    bbT = np.concatenate([b_re, b_im], axis=0).T  # (DM, 128)
    bb = np.ascontiguousarray(
        bbT.reshape(KC, P, P).transpose(1, 0, 2).reshape(P, DM)
    )
    bbh, bbl = _split_bf16(bb)
    ct = np.ascontiguousarray(np.asarray(C_mat, np.float32).T)  # (NS, DM)
    cth, ctl = _split_bf16(ct)
    t = np.arange(L, dtype=np.float64)
    ang = (im[:, None] * t[None, :]) % (2 * np.pi)
    cosT = np.cos(ang).astype(np.float32)
    sinT = np.sin(ang).astype(np.float32)
    trig = np.ascontiguousarray(np.concatenate([cosT, sinT], axis=0))  # (128, L)
    trigb = np.ascontiguousarray(np.concatenate([sinT, cosT], axis=0))
    rfull = np.concatenate([r, r]).astype(np.float32)
    rt = np.ascontiguousarray(np.broadcast_to(rfull[:, None], (P, LT)))
    return bbh, bbl, cth, ctl, trig, trigb, rt


_PROGRAM_CACHE = {}


def kernel(u, log_neg_real, imag, B_mat, C_mat, D, gamma, beta):
    _cache = _PROGRAM_CACHE
    u = np.ascontiguousarray(np.asarray(u, np.float32))
    Dv = np.asarray(D, np.float32)
    gam = np.asarray(gamma, np.float32)
    bet = np.asarray(beta, np.float32)
    use_ures = bool(np.any(Dv != 0.0))
    use_gb = bool(np.any(gam != 1.0) or np.any(bet != 0.0))

    bbh, bbl, cth, ctl, trig, trigb, rt = _host_params(
        log_neg_real, imag, B_mat, C_mat
    )

    key = (use_ures, use_gb)
    if key not in _cache:
        _cache[key] = _build_program(use_ures, use_gb)
    nc = _cache[key]

    shared = {
        "bbh": bbh,
        "bbl": bbl,
        "cth": cth,
        "ctl": ctl,
        "trig": trig,
        "trigb": trigb,
        "rt": rt,
    }
    if use_gb:
        shared["gam"] = np.ascontiguousarray(
            np.broadcast_to(gam[None, :], (P, DM)).astype(np.float32)
        )
        shared["bet"] = np.ascontiguousarray(
            np.broadcast_to(bet[None, :], (P, DM)).astype(np.float32)
        )
    in_maps = []
    for b in range(NCORES):
        m = dict(shared)
        m["u"] = np.ascontiguousarray(u[b])
        ut = np.ascontiguousarray(u[b].T)  # (DM, L)
        uth, utl = _split_bf16(ut)
        m["uth"] = np.ascontiguousarray(uth)
        m["utl"] = np.ascontiguousarray(utl)
        if use_ures:
            m["ures"] = np.ascontiguousarray(u[b] * (1.0 + Dv)[None, :])
        in_maps.append(m)

    res = bass_utils.run_bass_kernel_spmd(nc, in_maps, core_ids=list(range(NCORES)))
    return np.stack([r["out"] for r in res.results], axis=0)


# revision 19
# speedup vs baseline: 3.0449x; 1.2171x over previous
"""Trainium2 Bass kernel for the DiagonalSSMBlock problem.

Math (per batch, sharded one batch per core over 8 cores):
    a = -exp(log_neg_real) + i*imag ; a_bar = exp(a) = r * e^{i theta}
    b_bar = ((a_bar-1)/a)[:,None] * B
    Bu_t = b_bar @ u_t                         (complex, state dim 64)
    h_t = a_bar * h_{t-1} + Bu_t               (diagonal complex scan over L)
    y_t = Re(C @ h_t) + D*u_t ; out = LN(u + y) * gamma + beta

Device decomposition:
  * The Bu matmul contracts over d_model, so it consumes u in transposed
    layout. fp32 matmuls on the PE run in LOW_HIGH double-pass mode (~4.7x
    bf16 cost), so u is shipped as a host-precomputed transposed bf16 hi/lo
    pair and each matmul runs as 3 accumulating bf16 matmuls
    (hi*hi + lo*hi + hi*lo), recovering ~fp32 accuracy at bf16 speed.
  * Bu lands directly in scan layout [re|im states on 128 partitions, L free]
    via a packed [b_re; b_im]^T stationary operand.
  * The complex scan is rotated into a per-lane REAL damped scan:
    g_t = r*g_{t-1} + w_t with w_t = e^{-i theta t} Bu_t (elementwise
    rotation against host cos/sin tables), h_re_t = Re(e^{i theta t} g_t).
    The real scan maps to one DVE tensor_tensor_scan per 512-wide slice,
    chained via its initial value.
  * Readout y = h_re^T @ C^T (compensated bf16) packs two 128-row l-subtiles
    per step onto PE row-groups {0,1}/{2,3} (K=64 each) so the two matmuls
    run concurrently. Residual + LayerNorm fused on DVE/ACT:
    scalar_tensor_tensor computes x=y+u and accumulates sum(x), ACT Square
    accumulates sum(x^2), DVE tensor_scalar applies (x-mu)*rstd.
  * DMA is batched: one 2 MiB transfer per l-tile each for u / uth / utl /
    out via 3D access patterns (few large DMAs instead of many small ones).
"""

import numpy as np

import concourse.mybir as mybir
import concourse.tile as tile
from concourse import bacc, bass_utils
from concourse.bass import MemorySpace
from concourse.mybir import ActivationFunctionType as act
from concourse.mybir import AluOpType as alu

F32 = mybir.dt.float32
BF16 = mybir.dt.bfloat16
P = 128          # partitions
L = 4096         # sequence length per core
DM = 1024        # d_model
NS = 64          # d_state
LT = 512         # l-tile (scan slice, matmul moving width)
NSUB = LT // P   # 4 l-subtiles of 128 rows per l-tile
NT = L // LT     # 8 l-tiles
KC = DM // P     # 8 contraction chunks of 128
NCORES = 8
LN_EPS = 1e-5
DH = 512         # d-model half (psum bank width)


def _build_program(use_ures: bool, use_gb: bool):
    """Builds the single-core Bass/Tile program (SPMD across 8 cores).

    use_ures: residual uses a separate host-scaled input (when D != 0).
    use_gb:  apply gamma/beta via replicated tiles (when non-trivial).
    """
    nc = bacc.Bacc("TRN2", num_devices=NCORES, debug=False)

    u_d = nc.dram_tensor("u", [L, DM], F32, kind="ExternalInput").ap()
    uth_d = nc.dram_tensor("uth", [DM, L], BF16, kind="ExternalInput").ap()
    utl_d = nc.dram_tensor("utl", [DM, L], BF16, kind="ExternalInput").ap()
    bbh_d = nc.dram_tensor("bbh", [P, DM], BF16, kind="ExternalInput").ap()
    bbl_d = nc.dram_tensor("bbl", [P, DM], BF16, kind="ExternalInput").ap()
    ct2h_d = nc.dram_tensor("ct2h", [P, DM], BF16, kind="ExternalInput").ap()
    ct2l_d = nc.dram_tensor("ct2l", [P, DM], BF16, kind="ExternalInput").ap()
    trig_d = nc.dram_tensor("trig", [P, L], F32, kind="ExternalInput").ap()
    trigb_d = nc.dram_tensor("trigb", [P, L], F32, kind="ExternalInput").ap()
    rt_d = nc.dram_tensor("rt", [P, LT], F32, kind="ExternalInput").ap()
    ures_d = (
        nc.dram_tensor("ures", [L, DM], F32, kind="ExternalInput").ap()
        if use_ures
        else None
    )
    if use_gb:
        gam_d = nc.dram_tensor("gam", [P, DM], F32, kind="ExternalInput").ap()
        bet_d = nc.dram_tensor("bet", [P, DM], F32, kind="ExternalInput").ap()
    out_d = nc.dram_tensor("out", [L, DM], F32, kind="ExternalOutput").ap()

    # batched-DMA views: [p, s, d] with l = s*128 + p
    u_v = u_d.rearrange("(s p) d -> p s d", p=P)
    ur_v = ures_d.rearrange("(s p) d -> p s d", p=P) if use_ures else None
    out_v = out_d.rearrange("(s p) d -> p s d", p=P)
    # [p, c, l] with d = c*128 + p
    uth_v = uth_d.rearrange("(c p) l -> p c l", p=P)
    utl_v = utl_d.rearrange("(c p) l -> p c l", p=P)

    with tile.TileContext(nc) as tc:
        with (
            tc.tile_pool(name="singles", bufs=1) as singles,
            tc.tile_pool(name="u", bufs=2) as u_pool,
            tc.tile_pool(name="ur", bufs=2) as ur_pool,
            tc.tile_pool(name="ut", bufs=2) as ut_pool,
            tc.tile_pool(name="w", bufs=2) as w_pool,
            tc.tile_pool(name="g", bufs=3) as g_pool,
            tc.tile_pool(name="h", bufs=3) as h_pool,
            tc.tile_pool(name="x", bufs=5) as x_pool,
            tc.tile_pool(name="tmp", bufs=2) as tmp_pool,
            tc.tile_pool(name="o", bufs=2) as o_pool,
            tc.tile_pool(name="st", bufs=3) as st_pool,
            tc.tile_pool(name="pb", bufs=2, space=MemorySpace.PSUM) as psum_b,
            tc.tile_pool(name="py", bufs=3, space=MemorySpace.PSUM) as psum_y,
        ):
            bbh_s = singles.tile([P, DM], BF16)
            nc.sync.dma_start(bbh_s[:], bbh_d)
            bbl_s = singles.tile([P, DM], BF16)
            nc.sync.dma_start(bbl_s[:], bbl_d)
            ct2h_s = singles.tile([P, DM], BF16)
            nc.sync.dma_start(ct2h_s[:], ct2h_d)
            ct2l_s = singles.tile([P, DM], BF16)
            nc.sync.dma_start(ct2l_s[:], ct2l_d)
            trig = singles.tile([P, L], F32)
            nc.sync.dma_start(trig[:], trig_d)
            trigb = singles.tile([P, L], F32)
            nc.sync.dma_start(trigb[:], trigb_d)
            rt_s = singles.tile([P, LT], F32)
            nc.sync.dma_start(rt_s[:], rt_d)
            eps_s = singles.tile([P, 1], F32)
            nc.gpsimd.memset(eps_s[:], LN_EPS)
            if use_gb:
                gam_s = singles.tile([P, DM], F32)
                nc.sync.dma_start(gam_s[:], gam_d)
                bet_s = singles.tile([P, DM], F32)
                nc.sync.dma_start(bet_s[:], bet_d)

            g_prev = None
            for it in range(NT):
                l0 = it * LT
                u_t = u_pool.tile([P, NSUB, DM], F32, tag="u")
                nc.sync.dma_start(u_t[:], u_v[:, NSUB * it : NSUB * (it + 1), :])
                if use_ures:
                    ur_t = ur_pool.tile([P, NSUB, DM], F32, tag="ur")
                    nc.sync.dma_start(
                        ur_t[:], ur_v[:, NSUB * it : NSUB * (it + 1), :]
                    )
                else:
                    ur_t = u_t
                th_t = ut_pool.tile([P, KC, LT], BF16, tag="uth")
                nc.sync.dma_start(th_t[:], uth_v[:, :, l0 : l0 + LT])
                tl_t = ut_pool.tile([P, KC, LT], BF16, tag="utl")
                nc.sync.dma_start(tl_t[:], utl_v[:, :, l0 : l0 + LT])

                # --- Bu matmul over d-chunks, compensated bf16 ---
                bu = psum_b.tile([P, LT], F32, tag="bu")
                n_mm = 3 * KC
                mi = 0
                for k in range(KC):
                    for lhsT, rhs in (
                        (bbh_s, th_t),
                        (bbl_s, th_t),
                        (bbh_s, tl_t),
                    ):
                        nc.tensor.matmul(
                            bu[:],
                            lhsT[:, k * P : (k + 1) * P],
                            rhs[:, k, :],
                            start=(mi == 0),
                            stop=(mi == n_mm - 1),
                        )
                        mi += 1

                # trig: cos on parts 0-63, sin on 64-127; trigb: swapped halves.
                # SBUF+SBUF operands must share base partition (birverifier);
                # out may target either half.
                cs_lo = trig[0:NS, l0 : l0 + LT]
                sn_hi = trig[NS:P, l0 : l0 + LT]
                sn_lo = trigb[0:NS, l0 : l0 + LT]
                cs_hi = trigb[NS:P, l0 : l0 + LT]

                # --- pre-rotation: w = e^{-i theta t} * Bu  (DVE, SBUF-staged) ---
                bs = w_pool.tile([P, LT], F32, tag="bs")
                nc.scalar.copy(bs[:], bu[:])
                w = w_pool.tile([P, LT], F32, tag="w")
                t1 = tmp_pool.tile([NS, LT], F32, tag="t1")
                t2 = tmp_pool.tile([NS, LT], F32, tag="t2")
                nc.vector.tensor_tensor(t1[:], bs[0:NS, :], cs_lo, alu.mult)
                nc.vector.tensor_tensor(t2[:], bs[NS:P, :], sn_hi, alu.mult)
                nc.vector.tensor_tensor(w[0:NS, :], t1[:], t2[:], alu.add)
                t3 = tmp_pool.tile([NS, LT], F32, tag="t1")
                t4 = tmp_pool.tile([NS, LT], F32, tag="t2")
                nc.vector.tensor_tensor(t3[:], bs[NS:P, :], cs_hi, alu.mult)
                nc.vector.tensor_tensor(t4[:], bs[0:NS, :], sn_lo, alu.mult)
                nc.vector.tensor_tensor(w[NS:P, :], t3[:], t4[:], alu.subtract)

                # --- damped real scan (DVE), chained across l-tiles ---
                g = g_pool.tile([P, LT], F32, tag="g")
                init = 0.0 if g_prev is None else g_prev[:, LT - 1 : LT]
                nc.vector.tensor_tensor_scan(
                    g[:], rt_s[:], w[:], init, alu.mult, alu.add
                )
                g_prev = g

                # --- post-rotation: h_re = cos*g_re - sin*g_im  (POOL) ---
                # h2 pair tiles pack l-subs (2k, 2k+1) onto partition halves so
                # the readout can row-pack two K=64 matmuls onto PE row groups.
                t5 = tmp_pool.tile([NS, LT], F32, tag="t1")
                t6 = tmp_pool.tile([NS, LT], F32, tag="t2")
                nc.gpsimd.tensor_tensor(t5[:], g[0:NS, :], cs_lo, alu.mult)
                nc.gpsimd.tensor_tensor(t6[:], g[NS:P, :], sn_hi, alu.mult)
                hh_list = []
                hl_list = []
                for pr in range(NSUB // 2):
                    # cross-partition-half writes: DVE only (GpSimd cores are
                    # pinned to their native 16 partitions)
                    h2 = h_pool.tile([P, P], F32, tag="h2")
                    for half in range(2):
                        ls = 2 * pr + half
                        nc.vector.tensor_tensor(
                            h2[half * NS : (half + 1) * NS, :],
                            t5[:, ls * P : (ls + 1) * P],
                            t6[:, ls * P : (ls + 1) * P],
                            alu.subtract,
                        )
                    hh2 = h_pool.tile([P, P], BF16, tag="hh2")
                    nc.vector.tensor_copy(hh2[:], h2[:])
                    hl2 = h_pool.tile([P, P], BF16, tag="hl2")
                    nc.vector.scalar_tensor_tensor(
                        hl2[:], h2[:], 1.0, hh2[:], alu.mult, alu.subtract
                    )
                    hh_list.append(hh2)
                    hl_list.append(hl2)

                # --- readout (row-packed pairs) + residual + LN stats ---
                sx = st_pool.tile([P, 2 * NSUB], F32, tag="sx")
                sq = st_pool.tile([P, NSUB], F32, tag="sq")
                x_list = []
                for pr in range(NSUB // 2):
                    hh2 = hh_list[pr]
                    hl2 = hl_list[pr]
                    y_a = psum_y.tile([P, DM], F32, tag="y")
                    y_b = psum_y.tile([P, DM], F32, tag="y")
                    ys = [y_a, y_b]
                    for dh in range(2):
                        sl = slice(dh * DH, (dh + 1) * DH)
                        for si, (hcur, ctcur) in enumerate(
                            ((hh2, ct2h_s), (hl2, ct2h_s), (hh2, ct2l_s))
                        ):
                            for half in range(2):
                                hb = slice(half * NS, (half + 1) * NS)
                                nc.tensor.matmul(
                                    ys[half][:, sl],
                                    hcur[hb, :],
                                    ctcur[hb, sl],
                                    start=(si == 0),
                                    stop=(si == 2),
                                )
                    for half in range(2):
                        ls = 2 * pr + half
                        y = ys[half]
                        x = x_pool.tile([P, DM], F32, tag="x")
                        for dh in range(2):
                            sl = slice(dh * DH, (dh + 1) * DH)
                            nc.vector.scalar_tensor_tensor(
                                x[:, sl],
                                y[:, sl],
                                1.0,
                                ur_t[:, ls, sl],
                                alu.mult,
                                alu.add,
                                accum_out=sx[:, 2 * ls + dh : 2 * ls + dh + 1],
                            )
                        sqs = tmp_pool.tile([P, DM], F32, tag="sqs")
                        nc.scalar.activation(
                            sqs[:], x[:], act.Square, accum_out=sq[:, ls : ls + 1]
                        )
                        x_list.append(x)

                # --- LN stats for the 4 l-subtiles ---
                mu = st_pool.tile([P, NSUB], F32, tag="mu")
                nc.vector.tensor_tensor(
                    mu[:], sx[:, 0 : 2 * NSUB : 2], sx[:, 1 : 2 * NSUB : 2], alu.add
                )
                nc.scalar.mul(mu[:], mu[:], 1.0 / DM)
                ex2 = st_pool.tile([P, NSUB], F32, tag="ex2")
                nc.scalar.mul(ex2[:], sq[:], 1.0 / DM)
                var = st_pool.tile([P, NSUB], F32, tag="var")
                nc.vector.tensor_tensor(var[:], mu[:], mu[:], alu.mult)
                nc.vector.tensor_tensor(var[:], ex2[:], var[:], alu.subtract)
                sd = st_pool.tile([P, NSUB], F32, tag="sd")
                nc.scalar.activation(sd[:], var[:], act.Sqrt, bias=eps_s[:, 0:1])
                rstd = st_pool.tile([P, NSUB], F32, tag="rstd")
                nc.vector.reciprocal(rstd[:], sd[:])

                # --- normalize (DVE tensor_scalar) + batched store ---
                o_t = o_pool.tile([P, NSUB, DM], F32, tag="o")
                for ls in range(NSUB):
                    nc.vector.tensor_scalar(
                        o_t[:, ls, :],
                        x_list[ls][:],
                        mu[:, ls : ls + 1],
                        rstd[:, ls : ls + 1],
                        alu.subtract,
                        alu.mult,
                    )
                    if use_gb:
                        nc.vector.tensor_tensor(
                            o_t[:, ls, :], o_t[:, ls, :], gam_s[:], alu.mult
                        )
                        nc.vector.tensor_tensor(
                            o_t[:, ls, :], o_t[:, ls, :], bet_s[:], alu.add
                        )
                nc.sync.dma_start(out_v[:, NSUB * it : NSUB * (it + 1), :], o_t[:])
    nc.compile()
    return nc


try:
    import ml_dtypes

    ml_bf16 = ml_dtypes.bfloat16
except ImportError:  # pragma: no cover
    ml_bf16 = None


def _split_bf16(x):
    hi = x.astype(np.float32).astype(ml_bf16)
    lo = (x.astype(np.float32) - hi.astype(np.float32)).astype(ml_bf16)
    return hi, lo


def _host_params(log_neg_real, imag, B_mat, C_mat):
    lnr = np.asarray(log_neg_real, np.float64)
    im = np.asarray(imag, np.float64)
    a = -np.exp(lnr) + 1j * im
    a_bar = np.exp(a)
    r = np.abs(a_bar)
    b_bar = ((a_bar - 1.0) / a)[:, None] * np.asarray(B_mat, np.float64)
    b_re = np.real(b_bar).astype(np.float32)
    b_im = np.imag(b_bar).astype(np.float32)
    # packed stationary operand for the Bu matmul: [K=d, M=128(re|im)] laid out
    # in SBUF as [128 partitions, KC*128] with chunk k at columns k*128:(k+1)*128
    bbT = np.concatenate([b_re, b_im], axis=0).T  # (DM, 128)
    bb = np.ascontiguousarray(
        bbT.reshape(KC, P, P).transpose(1, 0, 2).reshape(P, DM)
    )
    bbh, bbl = _split_bf16(bb)
    ct = np.ascontiguousarray(np.asarray(C_mat, np.float32).T)  # (NS, DM)
    cth, ctl = _split_bf16(ct)
    # duplicated across both partition halves for the row-packed readout
    ct2h = np.ascontiguousarray(np.concatenate([cth, cth], axis=0))
    ct2l = np.ascontiguousarray(np.concatenate([ctl, ctl], axis=0))
    t = np.arange(L, dtype=np.float64)
    ang = (im[:, None] * t[None, :]) % (2 * np.pi)
    cosT = np.cos(ang).astype(np.float32)
    sinT = np.sin(ang).astype(np.float32)
    trig = np.ascontiguousarray(np.concatenate([cosT, sinT], axis=0))  # (128, L)
    trigb = np.ascontiguousarray(np.concatenate([sinT, cosT], axis=0))
    rfull = np.concatenate([r, r]).astype(np.float32)
    rt = np.ascontiguousarray(np.broadcast_to(rfull[:, None], (P, LT)))
    return bbh, bbl, ct2h, ct2l, trig, trigb, rt


_PROGRAM_CACHE = {}


def kernel(u, log_neg_real, imag, B_mat, C_mat, D, gamma, beta):
    _cache = _PROGRAM_CACHE
    u = np.ascontiguousarray(np.asarray(u, np.float32))
    Dv = np.asarray(D, np.float32)
    gam = np.asarray(gamma, np.float32)
    bet = np.asarray(beta, np.float32)
    use_ures = bool(np.any(Dv != 0.0))
    use_gb = bool(np.any(gam != 1.0) or np.any(bet != 0.0))

    bbh, bbl, ct2h, ct2l, trig, trigb, rt = _host_params(
        log_neg_real, imag, B_mat, C_mat
    )

    key = (use_ures, use_gb)
    if key not in _cache:
        _cache[key] = _build_program(use_ures, use_gb)
    nc = _cache[key]

    shared = {
        "bbh": bbh,
        "bbl": bbl,
        "ct2h": ct2h,
        "ct2l": ct2l,
        "trig": trig,
        "trigb": trigb,
        "rt": rt,
    }
    if use_gb:
        shared["gam"] = np.ascontiguousarray(
            np.broadcast_to(gam[None, :], (P, DM)).astype(np.float32)
        )
        shared["bet"] = np.ascontiguousarray(
            np.broadcast_to(bet[None, :], (P, DM)).astype(np.float32)
        )
    in_maps = []
    for b in range(NCORES):
        m = dict(shared)
        m["u"] = np.ascontiguousarray(u[b])
        ut = np.ascontiguousarray(u[b].T)  # (DM, L)
        uth, utl = _split_bf16(ut)
        m["uth"] = np.ascontiguousarray(uth)
        m["utl"] = np.ascontiguousarray(utl)
        if use_ures:
            m["ures"] = np.ascontiguousarray(u[b] * (1.0 + Dv)[None, :])
        in_maps.append(m)

    res = bass_utils.run_bass_kernel_spmd(nc, in_maps, core_ids=list(range(NCORES)))
    return np.stack([r["out"] for r in res.results], axis=0)
